# revision 17
# baseline (speedup 1.0000x reference)
"""Trainium2 Bass kernel for nn_Diffuser_78331613544465.

Math (per graph b of B=8, N=1024):
    A   = adj (mask all-ones in graded setup; general mask handled host-side)
    P   = A / max(rowsum(A), 1)
    out[i,j,:] = relu([I, P, P2, P4][i,j,:] @ w1 + b1) @ w2 + b2

Device strategy: data-parallel over B - one graph per NeuronCore (8 cores).

Key structure (A symmetric for undirected graphs):
  * M = D^-1/2 A D^-1/2 is symmetric, so M2 = M@M and M4 = M2@M2 need no PE
    transposes: every matmul lhsT tile is a direct slice of the symmetric
    operand.  P_s^T = D^1/2 M_s D^-1/2 is a cheap row+col scale.
  * Edge MLP layer 1 runs as four concurrent K=32 row-tiled matmuls
    (tile_position=(32r,0)), one j-block of 8 js per strip.  The self-stack
    (diagonal) is folded into the same matmul: weight rows 24..31 of each
    strip hold w1[0] and the rhs pad rows hold one-hot identity rows.
  * Layer 2 runs as two concurrent M=64 col-tiled matmuls
    (tile_position=(0,0)/(0,64)), two j-blocks per slot.
  * Output is written untransposed ((j,o)-major, i contiguous) in fp16 with
    2KB DMA chunks; the host permutes to [i,j,o] fp32.

kernel(**inputs) takes FULL inputs, shards over 8 cores, returns FULL output.
"""

import os
import numpy as np

B, N, P = 8, 1024, 128
HID, HEADS = 16, 8
NT = N // P          # 8 row-tiles
JBLK = 8             # j rows per MLP block
NJB = N // JBLK      # 128 j-blocks
IC = 512             # i-chunk (matmul free dim / PSUM bank)
NIC = N // IC        # 2
NHB = NJB // 2       # 64 MLP half-batches of 2 j-blocks (= 1 output pair)
NSLOT = 8            # ilbig ring slots (4 j-blocks per 2-slot gather granule)

_CACHE = {}
LAST_RESULTS = None


def _emit(nc, tc, ctx, mm_dt, b1_zero):
    from concourse import mybir

    f32 = mybir.dt.float32
    AT = mybir.AluOpType

    adj16 = nc.declare_dram_parameter("adj16", [N, N], mm_dt, isOutput=False)
    w1b4_d = nc.declare_dram_parameter("w1b4", [P, P], mm_dt, isOutput=False)
    w2b_d = nc.declare_dram_parameter("w2b", [P, JBLK * HEADS], mm_dt, isOutput=False)
    b1rep_d = nc.declare_dram_parameter("b1rep", [P, 1], f32, isOutput=False)
    idn16_d = nc.declare_dram_parameter("idn16", [P, P], mm_dt, isOutput=False)
    idn32_d = nc.declare_dram_parameter("idn32", [P, P], f32, isOutput=False)
    # output: outd[pair, 64*e + 8*jj + o, i] = out[i, 8*(2*pair+e)+jj, o]
    outd = nc.declare_dram_parameter("out", [NJB // 2, P, N], mm_dt, isOutput=True)

    from contextlib import ExitStack

    big = ctx.enter_context(tc.tile_pool(name="big", bufs=1))
    small = ctx.enter_context(tc.tile_pool(name="small", bufs=1))
    qst = ctx.enter_context(tc.tile_pool(name="qst", bufs=3))
    rtp = ctx.enter_context(tc.tile_pool(name="rtp", bufs=3))
    otp = ctx.enter_context(tc.tile_pool(name="otp", bufs=3))

    # persistent [128, NT*1024] matrices: tile t at free cols [1024t, 1024t+1024),
    # partition p = matrix row 128t+p
    scr = big.tile([P, NT * N], mm_dt, tag="scr")    # A16
    eyeb = big.tile([P, NT * N], mm_dt, tag="eyeb")  # identity
    M1 = big.tile([P, NT * N], mm_dt, tag="M1")
    M2 = big.tile([P, NT * N], mm_dt, tag="M2")
    ilbig = big.tile([P, NSLOT * N], mm_dt, tag="ilbig")

    # DRAM spill: per j-block jb, 32 rows: 8s+k = Q_{s}[8jb+k, :] for s in
    # {P1, P2, P4}; rows 24+k = one-hot e_{8jb+k}.  (+P pad rows so the
    # 2-batch gather rearrange slice stays in bounds.)
    dram = ctx.enter_context(tc.tile_pool(name="dram", bufs=1, space="DRAM"))
    QId = dram.tile([NJB * 32 + P, N], mm_dt, tag="QId")

    ph2 = ExitStack()
    mm2_ps = ph2.enter_context(tc.tile_pool(name="mm2_ps", bufs=4, space="PSUM"))
    ph1 = ExitStack()
    pt_ps = ph1.enter_context(tc.tile_pool(name="pt_ps", bufs=2, space="PSUM"))

    # ---- constants / weights --------------------------------------------
    idn32 = small.tile([P, P], f32, tag="idn32")
    nc.gpsimd.dma_start(idn32[:], idn32_d[:])
    idn16 = small.tile([P, P], mm_dt, tag="idn16")
    nc.gpsimd.dma_start(idn16[:], idn16_d[:])
    w1b4s = small.tile([P, P], mm_dt, tag="w1b4s")
    nc.gpsimd.dma_start(w1b4s[:], w1b4_d[:])
    w2bs = small.tile([P, JBLK * HEADS], mm_dt, tag="w2bs")
    nc.gpsimd.dma_start(w2bs[:], w2b_d[:])
    b1rep = small.tile([P, 1], f32, tag="b1rep")
    nc.gpsimd.dma_start(b1rep[:], b1rep_d[:])
    ones1 = small.tile([1, P], f32, tag="ones1")
    nc.vector.memset(ones1[:], 1.0)

    # ---- eye rows into QId (static content; emitted first so the spills
    # drain on the gpsimd queue during phase 1 / the squares) --------------
    for t in range(NT):
        nc.vector.memset(eyeb[:, N * t:N * (t + 1)], 0.0)
        nc.scalar.copy(eyeb[:, N * t + P * t:N * t + P * (t + 1)], idn16[:])
        dst = QId[:].rearrange("(jb z) c -> jb z c", z=32)[
            16 * t:16 * (t + 1), 24:32, :
        ]
        nc.gpsimd.dma_start(dst, eyeb[:, N * t:N * (t + 1)])

    # ---- phase 1: degree scalings ---------------------------------------
    dsq = small.tile([P, NT], f32, tag="dsq")    # d^{+1/2} per row of tile t
    dsi = small.tile([P, NT], f32, tag="dsi")    # d^{-1/2}
    for t in range(NT):
        eng = nc.gpsimd if t % 2 == 0 else nc.sync
        eng.dma_start(scr[:, N * t:N * (t + 1)], adj16[P * t:P * (t + 1), :])
        deg = small.tile([P, 1], f32, tag=f"deg{t}")
        nc.vector.tensor_reduce(
            deg[:], scr[:, N * t:N * (t + 1)],
            axis=mybir.AxisListType.X, op=AT.add,
        )
        degc = small.tile([P, 1], f32, tag=f"degc{t}")
        nc.vector.tensor_scalar_max(degc[:], deg[:], 1.0)
        nc.scalar.sqrt(dsq[:, t:t + 1], degc[:])
        nc.vector.reciprocal(dsi[:, t:t + 1], dsq[:, t:t + 1])

    # replicate d^{-1/2} across partitions: dsirep[p, i] = d_i^{-1/2}
    dsirow = small.tile([1, N], f32, tag="dsirow")
    for t in range(NT):
        ptp = pt_ps.tile([P, P], f32, tag="pt")
        nc.tensor.transpose(ptp[0:1, :], dsi[:, t:t + 1], idn32[:])
        nc.scalar.copy(dsirow[0:1, P * t:P * (t + 1)], ptp[0:1, :])
    dsirep = small.tile([P, N], mm_dt, tag="dsirep")
    invrep = small.tile([P, N], mm_dt, tag="invrep")
    for half in range(2):
        pb = mm2_ps.tile([P, IC], f32, tag="mm")
        nc.tensor.matmul(
            pb[:], ones1[:], dsirow[0:1, IC * half:IC * (half + 1)],
            start=True, stop=True,
        )
        nc.scalar.copy(dsirep[:, IC * half:IC * (half + 1)], pb[:])
    nc.vector.tensor_mul(invrep[:], dsirep[:], dsirep[:])  # d^{-1} replicated
    ph1.close()  # pt_ps banks free

    def spill(srcap, t, s):
        # write [128, 1024] SBUF tile (partition p = (pj k)) into QId rows
        # 32*(16t+pj) + 8s + k; flat element order matches.
        dst = QId[:].rearrange("(jb z) c -> jb z c", z=32)[
            16 * t:16 * (t + 1), 8 * s:8 * s + 8, :
        ]
        nc.gpsimd.dma_start(dst, srcap)

    # ---- Q1 = A * d^{-1} (col scale) ; M1 = d^{-1/2} A d^{-1/2} ----------
    for t in range(NT):
        q1 = qst.tile([P, N], mm_dt, tag="q1")
        nc.vector.tensor_mul(q1[:], scr[:, N * t:N * (t + 1)], invrep[:])
        spill(q1[:], t, 0)
        rs = qst.tile([P, N], mm_dt, tag="rs")
        nc.scalar.activation(
            rs[:], scr[:, N * t:N * (t + 1)],
            mybir.ActivationFunctionType.Copy, scale=dsi[:, t:t + 1],
        )
        nc.vector.tensor_mul(M1[:, N * t:N * (t + 1)], rs[:], dsirep[:])

    # ---- squares: X2 = X @ X with symmetric X (lhsT = direct slices) -----
    def square_tile(pool, src, al, be):
        mm = pool.tile([P, IC], f32, tag="mm", name="mm")
        for g in range(NT):
            nc.tensor.matmul(
                mm[:],
                src[:, N * g + P * al:N * g + P * (al + 1)],
                src[:, N * g + IC * be:N * g + IC * (be + 1)],
                start=(g == 0), stop=(g == NT - 1),
            )
        return mm

    # M2 = M1@M1; keep fp16 copy (needed for M4) + Q2 = d^1/2 M2 d^-1/2
    for al in range(NT):
        for be in range(NIC):
            mm = square_tile(mm2_ps, M1, al, be)
            dst = M2[:, N * al + IC * be:N * al + IC * (be + 1)]
            if (al + be) % 2 == 0:
                nc.scalar.copy(dst, mm[:])
            else:
                nc.vector.tensor_scalar_add(dst, mm[:], 0.0)
        if al % 2 == 1:
            for t in (al - 1, al):
                rs = qst.tile([P, N], mm_dt, tag="rs")
                nc.scalar.activation(
                    rs[:], M2[:, N * t:N * (t + 1)],
                    mybir.ActivationFunctionType.Copy, scale=dsq[:, t:t + 1],
                )
                q2 = qst.tile([P, N], mm_dt, tag="q2")
                nc.vector.tensor_mul(q2[:], rs[:], dsirep[:])
                spill(q2[:], t, 1)

    ph2.close()
    mm4_ps = ctx.enter_context(tc.tile_pool(name="mm4_ps", bufs=2, space="PSUM"))
    h_ps = ctx.enter_context(tc.tile_pool(name="h_ps", bufs=2, space="PSUM"))
    po_ps = ctx.enter_context(tc.tile_pool(name="po_ps", bufs=2, space="PSUM"))

    relu = mybir.ActivationFunctionType.Relu
    copyf = mybir.ActivationFunctionType.Copy

    def gather(q):
        # fill slots (2q%NSLOT, +1) with j-blocks 8q..8q+8: strip r gets rows
        # 32*jb .. 32*jb+32 for jb = 8q+r and jb = 8q+4+r (J dim)
        for r in range(4):
            jb0 = 8 * q + r
            src = QId[32 * jb0:32 * jb0 + 2 * P, :].rearrange(
                "(J z) c -> z J c", z=P
            )[0:32, :, :]
            s0 = 2 * q % NSLOT
            nc.sync.dma_start(
                ilbig[32 * r:32 * r + 32, s0 * N:(s0 + 2) * N], src
            )

    def halfbatch(hb):
        # 2 j-blocks (jb0=2hb, jb0+1) = output pair hb
        jb0 = 2 * hb
        slot = (hb // 2) % NSLOT
        r0 = jb0 % 4
        hs = []
        for ic in range(NIC):
            h = h_ps.tile([P, N], f32, tag="h", name="h")
            for j in range(2):
                r = r0 + j
                nc.tensor.matmul(
                    h[:, IC * j:IC * (j + 1)],
                    w1b4s[32 * r:32 * r + 32, :],
                    ilbig[32 * r:32 * r + 32,
                          slot * N + IC * ic:slot * N + IC * (ic + 1)],
                    start=True, stop=True, tile_position=(32 * r, 0),
                    skip_group_check=True,
                )
            hs.append(h)
        rts = []
        for ic in range(NIC):
            rt = rtp.tile([P, N], mm_dt, tag="rt", name="rt")
            if (hb + ic) % 2 == 0:
                nc.scalar.activation(rt[:], hs[ic][:], relu, bias=b1rep[:], scale=1.0)
            elif b1_zero:
                nc.vector.tensor_scalar_max(rt[:], hs[ic][:], 0.0)
            else:
                nc.vector.tensor_scalar(rt[:], hs[ic][:], b1rep[:], 0.0, AT.add, AT.max)
            rts.append(rt)
        ot = otp.tile([P, N], mm_dt, tag="ot", name="ot")
        for ic in range(NIC):
            po = po_ps.tile([P, IC], f32, tag="po", name="po")
            for e in range(2):
                nc.tensor.matmul(
                    po[64 * e:64 * e + 64, :],
                    w2bs[:],
                    rts[ic][:, IC * e:IC * (e + 1)],
                    start=True, stop=True, tile_position=(0, 64 * e),
                    skip_group_check=True,
                )
            if (hb + ic) % 2 == 0:
                nc.vector.tensor_scalar_add(ot[:, IC * ic:IC * (ic + 1)], po[:], 0.0)
            else:
                nc.scalar.activation(ot[:, IC * ic:IC * (ic + 1)], po[:], copyf)
        eng = nc.sync if hb % 2 == 0 else nc.gpsimd
        eng.dma_start(outd[hb, :, :], ot[:])

    # ---- M4 = M2@M2 fused into Q4, interleaved with the MLP (half-batches
    # for row-tile al-2 run while al's square streams) ----------------------
    LAG = 2
    for al in range(NT):
        for be in range(NIC):
            mm = square_tile(mm4_ps, M2, al, be)
            q4 = qst.tile([P, IC], mm_dt, tag="q4", name="q4")
            nc.vector.tensor_mul(q4[:], mm[:], dsirep[:, IC * be:IC * (be + 1)])
            q4s = qst.tile([P, IC], mm_dt, tag="q4s", name="q4s")
            nc.scalar.activation(
                q4s[:], q4[:], copyf, scale=dsq[:, al:al + 1],
            )
            dst = QId[:].rearrange("(jb z) c -> jb z c", z=32)[
                16 * al:16 * (al + 1), 16:24, IC * be:IC * (be + 1)
            ]
            nc.scalar.dma_start(dst, q4s[:])
        if al >= LAG:
            for hb in range(8 * (al - LAG), 8 * (al - LAG) + 8):
                halfbatch(hb)
        # emit gathers AFTER the half-batches that still read the slots
        # these overwrite (ring WAR ordering in the Tile dep tracker)
        gather(2 * al)
        gather(2 * al + 1)
    for hb in range(8 * (NT - LAG), NHB):
        halfbatch(hb)


def _build(mm_dtype_name="float16", b1_zero=True):
    key = (mm_dtype_name, b1_zero)
    if key in _CACHE:
        return _CACHE[key]
    from contextlib import ExitStack
    import concourse.tile as tile
    from concourse import bacc, mybir

    nc = bacc.Bacc()
    with tile.TileContext(nc) as tc:
        with ExitStack() as ctx:
            _emit(nc, tc, ctx, getattr(mybir.dt, mm_dtype_name), b1_zero)
    nc.compile()
    _CACHE[key] = nc
    return nc


def _install_ntff_shim():
    """The agent image's antenv lacks axon_hooks; provide it and register the
    ctypes NTFF hook so run_bass_kernel_spmd(trace=True) can profile."""
    import sys
    import types

    if "antenv.axon_hooks" in sys.modules:
        return
    mod = types.ModuleType("antenv.axon_hooks")
    mod._hook = None
    mod.set_axon_ntff_profile_hook = lambda h: setattr(mod, "_hook", h)
    mod.get_axon_ntff_profile_hook = lambda: mod._hook
    sys.modules["antenv.axon_hooks"] = mod
    try:
        from trn_agent_boot.trn_boot import _ntff_profile_via_ctypes

        mod._hook = _ntff_profile_via_ctypes("/opt/axon/libaxon_pjrt.so")
    except Exception as e:  # degrade to no-trace
        print(f"ntff shim install failed: {e}")


def kernel(adj, mask, w1, b1, w2, b2):
    from concourse.bass_utils import run_bass_kernel_spmd

    global LAST_RESULTS
    adj = np.asarray(adj, dtype=np.float32)
    mask = np.asarray(mask)
    w1 = np.ascontiguousarray(np.asarray(w1, dtype=np.float32))
    b1 = np.ascontiguousarray(np.asarray(b1, dtype=np.float32))
    w2 = np.ascontiguousarray(np.asarray(w2, dtype=np.float32))
    b2 = np.asarray(b2, dtype=np.float32)
    assert adj.shape == (B, N, N), adj.shape

    m = mask.astype(np.float32)
    general_mask = not np.all(m == 1.0)
    if general_mask:
        pair = m[:, :, None] * m[:, None, :]
        adj = adj * pair

    trace = bool(int(os.environ.get("KERNEL_TRACE", "0")))
    if trace:
        _install_ntff_shim()
    mmname = os.environ.get("KERNEL_MM_DT", "float16")
    nc = _build(mmname, b1_zero=not bool(np.any(b1 != 0.0)))

    from concourse import mybir

    np_mm = mybir.dt.np(getattr(mybir.dt, mmname))

    # weights: w1b4[32r + 8s + k, 16k:16k+16] = w1[s+1]; rows 32r+24+k = w1[0]
    w1b4_np = np.zeros((P, P), np.float32)
    for r in range(4):
        for k in range(JBLK):
            for s in range(3):
                w1b4_np[32 * r + 8 * s + k, HID * k:HID * (k + 1)] = w1[s + 1]
            w1b4_np[32 * r + 24 + k, HID * k:HID * (k + 1)] = w1[0]
    # w2b[16jj + h, 8jj + o] = w2[h, o]
    w2b_np = np.zeros((P, JBLK * HEADS), np.float32)
    for jj in range(JBLK):
        w2b_np[HID * jj:HID * (jj + 1), HEADS * jj:HEADS * (jj + 1)] = w2

    shared = {
        "w1b4": w1b4_np.astype(np_mm),
        "w2b": w2b_np.astype(np_mm),
        "b1rep": np.ascontiguousarray(np.tile(b1, JBLK).astype(np.float32)[:, None]),
        "idn16": np.eye(P, dtype=np_mm),
        "idn32": np.eye(P, dtype=np.float32),
    }
    in_maps = [
        {"adj16": np.ascontiguousarray(adj[c].astype(np_mm)), **shared}
        for c in range(B)
    ]
    res = run_bass_kernel_spmd(nc, in_maps, list(range(B)), trace=trace)
    LAST_RESULTS = res

    # outd[pair, 64e + 8jj + o, i] -> out[i, 8(2 pair + e) + jj, o]
    outp = np.empty((B, N, N, HEADS), np.float32)
    for c in range(B):
        od = res.results[c]["out"].reshape(NJB // 2, 2, JBLK, HEADS, N)
        outp[c] = od.transpose(4, 0, 1, 2, 3).reshape(N, N, HEADS).astype(np.float32)

    if np.any(b2 != 0.0):
        outp = outp + b2
    if general_mask:
        outp = outp * pair[..., None]
    return np.ascontiguousarray(outp)


# revision 20
# speedup vs baseline: 1.2214x; 1.2214x over previous
"""Trainium2 Bass kernel for nn_Diffuser_78331613544465.

Math (per graph b of B=8, N=1024):
    A   = adj (mask all-ones in graded setup; general mask handled host-side)
    P   = A / max(rowsum(A), 1)
    out[i,j,:] = relu([I, P, P2, P4][i,j,:] @ w1 + b1) @ w2 + b2

Device strategy: data-parallel over B - one graph per NeuronCore (8 cores).

Key structure (A symmetric for undirected graphs):
  * M = D^-1/2 A D^-1/2 is symmetric, so M2 = M@M and M4 = M2@M2 need no PE
    transposes: every matmul lhsT tile is a direct slice of the symmetric
    operand.  P_s^T = D^1/2 M_s D^-1/2 is a cheap row+col scale.
  * Edge MLP layer 1 runs as four concurrent K=32 row-tiled matmuls
    (tile_position=(32r,0)), one j-block of 8 js per strip.  The self-stack
    (diagonal) is folded into the same matmul: weight rows 24..31 of each
    strip hold w1[0] and the rhs pad rows hold one-hot identity rows.
  * Layer 2 runs as two concurrent M=64 col-tiled matmuls
    (tile_position=(0,0)/(0,64)), two j-blocks per slot.
  * Output is written untransposed ((j,o)-major, i contiguous) in fp16 with
    2KB DMA chunks; the host permutes to [i,j,o] fp32.

kernel(**inputs) takes FULL inputs, shards over 8 cores, returns FULL output.
"""

import os
import numpy as np

B, N, P = 8, 1024, 128
HID, HEADS = 16, 8
NT = N // P          # 8 row-tiles
JBLK = 8             # j rows per MLP block
NJB = N // JBLK      # 128 j-blocks
IC = 512             # i-chunk (matmul free dim / PSUM bank)
NIC = N // IC        # 2
NHB = NJB // 2       # 64 MLP half-batches of 2 j-blocks (= 1 output pair)
NSLOT = 8            # ilbig ring slots (4 j-blocks per 2-slot gather granule)

_CACHE = {}
LAST_RESULTS = None


def _emit(nc, tc, ctx, mm_dt, b1_zero):
    from concourse import mybir

    f32 = mybir.dt.float32
    AT = mybir.AluOpType

    adj16 = nc.declare_dram_parameter("adj16", [N, N], mm_dt, isOutput=False)
    w1b4_d = nc.declare_dram_parameter("w1b4", [P, P], mm_dt, isOutput=False)
    w2b_d = nc.declare_dram_parameter("w2b", [P, JBLK * HEADS], mm_dt, isOutput=False)
    b1rep_d = nc.declare_dram_parameter("b1rep", [P, 1], f32, isOutput=False)
    idn16_d = nc.declare_dram_parameter("idn16", [P, P], mm_dt, isOutput=False)
    idn32_d = nc.declare_dram_parameter("idn32", [P, P], f32, isOutput=False)
    # output: outd[pair, 64*e + 8*jj + o, i] = out[i, 8*(2*pair+e)+jj, o]
    outd = nc.declare_dram_parameter("out", [NJB // 2, P, N], mm_dt, isOutput=True)

    from contextlib import ExitStack

    big = ctx.enter_context(tc.tile_pool(name="big", bufs=1))
    small = ctx.enter_context(tc.tile_pool(name="small", bufs=1))
    qst = ctx.enter_context(tc.tile_pool(name="qst", bufs=3))
    rtp = ctx.enter_context(tc.tile_pool(name="rtp", bufs=4))
    otp = ctx.enter_context(tc.tile_pool(name="otp", bufs=3))

    # persistent [128, NT*1024] matrices: tile t at free cols [1024t, 1024t+1024),
    # partition p = matrix row 128t+p
    scr = big.tile([P, NT * N], mm_dt, tag="scr")    # A16
    eyeb = big.tile([P, NT * N], mm_dt, tag="eyeb")  # identity
    M1 = big.tile([P, NT * N], mm_dt, tag="M1")
    M2 = big.tile([P, NT * N], mm_dt, tag="M2")
    ilbig = big.tile([P, NSLOT * N], mm_dt, tag="ilbig")

    # DRAM spill: per j-block jb, 32 rows: 8s+k = Q_{s}[8jb+k, :] for s in
    # {P1, P2, P4}; rows 24+k = one-hot e_{8jb+k}.  (+P pad rows so the
    # 2-batch gather rearrange slice stays in bounds.)
    dram = ctx.enter_context(tc.tile_pool(name="dram", bufs=1, space="DRAM"))
    QId = dram.tile([NJB * 32 + P, N], mm_dt, tag="QId")

    ph2 = ExitStack()
    mm2_ps = ph2.enter_context(tc.tile_pool(name="mm2_ps", bufs=4, space="PSUM"))
    ph1 = ExitStack()
    pt_ps = ph1.enter_context(tc.tile_pool(name="pt_ps", bufs=2, space="PSUM"))

    # ---- constants / weights --------------------------------------------
    idn32 = small.tile([P, P], f32, tag="idn32")
    nc.gpsimd.dma_start(idn32[:], idn32_d[:])
    idn16 = small.tile([P, P], mm_dt, tag="idn16")
    nc.gpsimd.dma_start(idn16[:], idn16_d[:])
    w1b4s = small.tile([P, P], mm_dt, tag="w1b4s")
    nc.gpsimd.dma_start(w1b4s[:], w1b4_d[:])
    w2bs = small.tile([P, JBLK * HEADS], mm_dt, tag="w2bs")
    nc.gpsimd.dma_start(w2bs[:], w2b_d[:])
    b1rep = small.tile([P, 1], f32, tag="b1rep")
    nc.gpsimd.dma_start(b1rep[:], b1rep_d[:])
    ones1 = small.tile([1, P], f32, tag="ones1")
    nc.vector.memset(ones1[:], 1.0)

    # ---- eye rows into QId (static content; emitted first so the spills
    # drain on the gpsimd queue during phase 1 / the squares) --------------
    for t in range(NT):
        nc.vector.memset(eyeb[:, N * t:N * (t + 1)], 0.0)
        nc.scalar.copy(eyeb[:, N * t + P * t:N * t + P * (t + 1)], idn16[:])
        dst = QId[:].rearrange("(jb z) c -> jb z c", z=32)[
            16 * t:16 * (t + 1), 24:32, :
        ]
        nc.gpsimd.dma_start(dst, eyeb[:, N * t:N * (t + 1)])

    # ---- phase 1: degree scalings ---------------------------------------
    dsq = small.tile([P, NT], f32, tag="dsq")    # d^{+1/2} per row of tile t
    dsi = small.tile([P, NT], f32, tag="dsi")    # d^{-1/2}
    for t in range(NT):
        eng = nc.gpsimd if t % 2 == 0 else nc.sync
        eng.dma_start(scr[:, N * t:N * (t + 1)], adj16[P * t:P * (t + 1), :])
        deg = small.tile([P, 1], f32, tag=f"deg{t}")
        nc.vector.tensor_reduce(
            deg[:], scr[:, N * t:N * (t + 1)],
            axis=mybir.AxisListType.X, op=AT.add,
        )
        degc = small.tile([P, 1], f32, tag=f"degc{t}")
        nc.vector.tensor_scalar_max(degc[:], deg[:], 1.0)
        nc.scalar.sqrt(dsq[:, t:t + 1], degc[:])
        nc.vector.reciprocal(dsi[:, t:t + 1], dsq[:, t:t + 1])

    # replicate d^{-1/2} across partitions: dsirep[p, i] = d_i^{-1/2}
    dsirow = small.tile([1, N], f32, tag="dsirow")
    for t in range(NT):
        ptp = pt_ps.tile([P, P], f32, tag="pt")
        nc.tensor.transpose(ptp[0:1, :], dsi[:, t:t + 1], idn32[:])
        nc.scalar.copy(dsirow[0:1, P * t:P * (t + 1)], ptp[0:1, :])
    dsirep = small.tile([P, N], mm_dt, tag="dsirep")
    invrep = small.tile([P, N], mm_dt, tag="invrep")
    for half in range(2):
        pb = mm2_ps.tile([P, IC], f32, tag="mm")
        nc.tensor.matmul(
            pb[:], ones1[:], dsirow[0:1, IC * half:IC * (half + 1)],
            start=True, stop=True,
        )
        nc.scalar.copy(dsirep[:, IC * half:IC * (half + 1)], pb[:])
    nc.vector.tensor_mul(invrep[:], dsirep[:], dsirep[:])  # d^{-1} replicated
    ph1.close()  # pt_ps banks free

    def spill(srcap, t, s):
        # write [128, 1024] SBUF tile (partition p = (pj k)) into QId rows
        # 32*(16t+pj) + 8s + k; flat element order matches.
        dst = QId[:].rearrange("(jb z) c -> jb z c", z=32)[
            16 * t:16 * (t + 1), 8 * s:8 * s + 8, :
        ]
        nc.gpsimd.dma_start(dst, srcap)

    # ---- Q1 = A * d^{-1} (col scale) ; M1 = d^{-1/2} A d^{-1/2} ----------
    for t in range(NT):
        q1 = qst.tile([P, N], mm_dt, tag="q1")
        nc.vector.tensor_mul(q1[:], scr[:, N * t:N * (t + 1)], invrep[:])
        spill(q1[:], t, 0)
        rs = qst.tile([P, N], mm_dt, tag="rs")
        nc.scalar.activation(
            rs[:], scr[:, N * t:N * (t + 1)],
            mybir.ActivationFunctionType.Copy, scale=dsi[:, t:t + 1],
        )
        nc.vector.tensor_mul(M1[:, N * t:N * (t + 1)], rs[:], dsirep[:])

    # ---- squares: X2 = X @ X with symmetric X (lhsT = direct slices) -----
    def square_tile(pool, src, al, be):
        mm = pool.tile([P, IC], f32, tag="mm", name="mm")
        for g in range(NT):
            nc.tensor.matmul(
                mm[:],
                src[:, N * g + P * al:N * g + P * (al + 1)],
                src[:, N * g + IC * be:N * g + IC * (be + 1)],
                start=(g == 0), stop=(g == NT - 1),
            )
        return mm

    # M2 = M1@M1; keep fp16 copy (needed for M4) + Q2 = d^1/2 M2 d^-1/2
    for al in range(NT):
        for be in range(NIC):
            mm = square_tile(mm2_ps, M1, al, be)
            dst = M2[:, N * al + IC * be:N * al + IC * (be + 1)]
            if (al + be) % 2 == 0:
                nc.scalar.copy(dst, mm[:])
            else:
                nc.vector.tensor_scalar_add(dst, mm[:], 0.0)
        if al % 2 == 1:
            for t in (al - 1, al):
                rs = qst.tile([P, N], mm_dt, tag="rs")
                nc.scalar.activation(
                    rs[:], M2[:, N * t:N * (t + 1)],
                    mybir.ActivationFunctionType.Copy, scale=dsq[:, t:t + 1],
                )
                q2 = qst.tile([P, N], mm_dt, tag="q2")
                nc.vector.tensor_mul(q2[:], rs[:], dsirep[:])
                spill(q2[:], t, 1)

    ph2.close()
    mm4_ps = ctx.enter_context(tc.tile_pool(name="mm4_ps", bufs=2, space="PSUM"))
    h_ps = ctx.enter_context(tc.tile_pool(name="h_ps", bufs=2, space="PSUM"))
    po_ps = ctx.enter_context(tc.tile_pool(name="po_ps", bufs=2, space="PSUM"))

    relu = mybir.ActivationFunctionType.Relu
    copyf = mybir.ActivationFunctionType.Copy

    def gather(q):
        # fill slots (2q%NSLOT, +1) with j-blocks 8q..8q+8: strip r gets rows
        # 32*jb .. 32*jb+32 for jb = 8q+r and jb = 8q+4+r (J dim)
        for r in range(4):
            jb0 = 8 * q + r
            src = QId[32 * jb0:32 * jb0 + 2 * P, :].rearrange(
                "(J z) c -> z J c", z=P
            )[0:32, :, :]
            s0 = 2 * q % NSLOT
            nc.sync.dma_start(
                ilbig[32 * r:32 * r + 32, s0 * N:(s0 + 2) * N], src
            )

    def hb_front(hb):
        # layer 1 + relu for 2 j-blocks (jb0=2hb, jb0+1); returns rt tiles
        jb0 = 2 * hb
        slot = (hb // 2) % NSLOT
        r0 = jb0 % 4
        hs = []
        for ic in range(NIC):
            h = h_ps.tile([P, N], f32, tag="h", name="h")
            for j in range(2):
                r = r0 + j
                nc.tensor.matmul(
                    h[:, IC * j:IC * (j + 1)],
                    w1b4s[32 * r:32 * r + 32, :],
                    ilbig[32 * r:32 * r + 32,
                          slot * N + IC * ic:slot * N + IC * (ic + 1)],
                    start=True, stop=True, tile_position=(32 * r, 0),
                    skip_group_check=True,
                )
            hs.append(h)
        rts = []
        for ic in range(NIC):
            rt = rtp.tile([P, N], mm_dt, tag="rt", name="rt")
            if (hb + ic) % 2 == 0:
                nc.scalar.activation(rt[:], hs[ic][:], relu, bias=b1rep[:], scale=1.0)
            elif b1_zero:
                nc.vector.tensor_scalar_max(rt[:], hs[ic][:], 0.0)
            else:
                nc.vector.tensor_scalar(rt[:], hs[ic][:], b1rep[:], 0.0, AT.add, AT.max)
            rts.append(rt)
        return rts

    def hb_back(hb, rts):
        # layer 2 + output evac for half-batch hb (emitted one hb later so
        # the PE FIFO has layer-1 work to run while this hb's relu finishes)
        ot = otp.tile([P, N], mm_dt, tag="ot", name="ot")
        for ic in range(NIC):
            po = po_ps.tile([P, IC], f32, tag="po", name="po")
            for e in range(2):
                nc.tensor.matmul(
                    po[64 * e:64 * e + 64, :],
                    w2bs[:],
                    rts[ic][:, IC * e:IC * (e + 1)],
                    start=True, stop=True, tile_position=(0, 64 * e),
                    skip_group_check=True,
                )
            if (hb + ic) % 2 == 0:
                nc.vector.tensor_scalar_add(ot[:, IC * ic:IC * (ic + 1)], po[:], 0.0)
            else:
                nc.scalar.activation(ot[:, IC * ic:IC * (ic + 1)], po[:], copyf)
        eng = nc.sync if hb % 2 == 0 else nc.gpsimd
        eng.dma_start(outd[hb, :, :], ot[:])

    pend = [None, None]  # (hb, rts) of the half-batch awaiting its back half

    def halfbatch(hb):
        rts = hb_front(hb)
        if pend[0] is not None:
            hb_back(pend[0], pend[1])
        pend[0], pend[1] = hb, rts

    # ---- M4 = M2@M2 fused into Q4, interleaved with the MLP (half-batches
    # for row-tile al-2 run while al's square streams) ----------------------
    LAG = 2
    for al in range(NT):
        for be in range(NIC):
            mm = square_tile(mm4_ps, M2, al, be)
            q4 = qst.tile([P, IC], mm_dt, tag="q4", name="q4")
            nc.vector.tensor_mul(q4[:], mm[:], dsirep[:, IC * be:IC * (be + 1)])
            q4s = qst.tile([P, IC], mm_dt, tag="q4s", name="q4s")
            nc.scalar.activation(
                q4s[:], q4[:], copyf, scale=dsq[:, al:al + 1],
            )
            dst = QId[:].rearrange("(jb z) c -> jb z c", z=32)[
                16 * al:16 * (al + 1), 16:24, IC * be:IC * (be + 1)
            ]
            nc.scalar.dma_start(dst, q4s[:])
        if al >= LAG:
            for hb in range(8 * (al - LAG), 8 * (al - LAG) + 8):
                halfbatch(hb)
        # emit gathers AFTER the half-batches that still read the slots
        # these overwrite (ring WAR ordering in the Tile dep tracker)
        gather(2 * al)
        gather(2 * al + 1)
    for hb in range(8 * (NT - LAG), NHB):
        halfbatch(hb)
    hb_back(pend[0], pend[1])


def _build(mm_dtype_name="float16", b1_zero=True):
    key = (mm_dtype_name, b1_zero)
    if key in _CACHE:
        return _CACHE[key]
    from contextlib import ExitStack
    import concourse.tile as tile
    from concourse import bacc, mybir

    nc = bacc.Bacc()
    with tile.TileContext(nc) as tc:
        with ExitStack() as ctx:
            _emit(nc, tc, ctx, getattr(mybir.dt, mm_dtype_name), b1_zero)
    nc.compile()
    _CACHE[key] = nc
    return nc


def _install_ntff_shim():
    """The agent image's antenv lacks axon_hooks; provide it and register the
    ctypes NTFF hook so run_bass_kernel_spmd(trace=True) can profile."""
    import sys
    import types

    if "antenv.axon_hooks" in sys.modules:
        return
    mod = types.ModuleType("antenv.axon_hooks")
    mod._hook = None
    mod.set_axon_ntff_profile_hook = lambda h: setattr(mod, "_hook", h)
    mod.get_axon_ntff_profile_hook = lambda: mod._hook
    sys.modules["antenv.axon_hooks"] = mod
    try:
        from trn_agent_boot.trn_boot import _ntff_profile_via_ctypes

        mod._hook = _ntff_profile_via_ctypes("/opt/axon/libaxon_pjrt.so")
    except Exception as e:  # degrade to no-trace
        print(f"ntff shim install failed: {e}")


def kernel(adj, mask, w1, b1, w2, b2):
    from concourse.bass_utils import run_bass_kernel_spmd

    global LAST_RESULTS
    adj = np.asarray(adj, dtype=np.float32)
    mask = np.asarray(mask)
    w1 = np.ascontiguousarray(np.asarray(w1, dtype=np.float32))
    b1 = np.ascontiguousarray(np.asarray(b1, dtype=np.float32))
    w2 = np.ascontiguousarray(np.asarray(w2, dtype=np.float32))
    b2 = np.asarray(b2, dtype=np.float32)
    assert adj.shape == (B, N, N), adj.shape

    m = mask.astype(np.float32)
    general_mask = not np.all(m == 1.0)
    if general_mask:
        pair = m[:, :, None] * m[:, None, :]
        adj = adj * pair

    trace = bool(int(os.environ.get("KERNEL_TRACE", "0")))
    if trace:
        _install_ntff_shim()
    mmname = os.environ.get("KERNEL_MM_DT", "float16")
    nc = _build(mmname, b1_zero=not bool(np.any(b1 != 0.0)))

    from concourse import mybir

    np_mm = mybir.dt.np(getattr(mybir.dt, mmname))

    # weights: w1b4[32r + 8s + k, 16k:16k+16] = w1[s+1]; rows 32r+24+k = w1[0]
    w1b4_np = np.zeros((P, P), np.float32)
    for r in range(4):
        for k in range(JBLK):
            for s in range(3):
                w1b4_np[32 * r + 8 * s + k, HID * k:HID * (k + 1)] = w1[s + 1]
            w1b4_np[32 * r + 24 + k, HID * k:HID * (k + 1)] = w1[0]
    # w2b[16jj + h, 8jj + o] = w2[h, o]
    w2b_np = np.zeros((P, JBLK * HEADS), np.float32)
    for jj in range(JBLK):
        w2b_np[HID * jj:HID * (jj + 1), HEADS * jj:HEADS * (jj + 1)] = w2

    shared = {
        "w1b4": w1b4_np.astype(np_mm),
        "w2b": w2b_np.astype(np_mm),
        "b1rep": np.ascontiguousarray(np.tile(b1, JBLK).astype(np.float32)[:, None]),
        "idn16": np.eye(P, dtype=np_mm),
        "idn32": np.eye(P, dtype=np.float32),
    }
    in_maps = [
        {"adj16": np.ascontiguousarray(adj[c].astype(np_mm)), **shared}
        for c in range(B)
    ]
    res = run_bass_kernel_spmd(nc, in_maps, list(range(B)), trace=trace)
    LAST_RESULTS = res

    # outd[pair, 64e + 8jj + o, i] -> out[i, 8(2 pair + e) + jj, o]
    outp = np.empty((B, N, N, HEADS), np.float32)
    for c in range(B):
        od = res.results[c]["out"].reshape(NJB // 2, 2, JBLK, HEADS, N)
        outp[c] = od.transpose(4, 0, 1, 2, 3).reshape(N, N, HEADS).astype(np.float32)

    if np.any(b2 != 0.0):
        outp = outp + b2
    if general_mask:
        outp = outp * pair[..., None]
    return np.ascontiguousarray(outp)
